# revision 15
# baseline (speedup 1.0000x reference)
"""ComPosHGNN Trainium2 kernel v3: 4-relation heterogeneous GraphConv.

Sharding: edges bucketed by destination range (5000 dst/core, both
ntypes per core), no collectives.  Host work is layout only (bucket/
sort/pad/replicate/dtype-cast); all arithmetic (degree sums,
normalization, projection, aggregation, relu) runs on device.

v3: no device gather.  The host replicates raw bf16 source rows into a
partition-major [128, NB*128] stream per relation (slot p of block b =
edge b*128+p), so the kernel reads them as large contiguous HWDGE DMAs.
Per-edge out-degree sums are computed on device from a replicated
neighbor-weight layout [128, NB*P2] (reduce_sum -> rsqrt) and folded
into the one-hot weights; in-degree rsqrt is applied per output tile.
Aggregation G^T[d,dst] += sum_e row[e,d] * oh[e,dst] accumulates in
PSUM via TensorE; the projection W uses lhsT=W with no transposes.
One-hot builds alternate between VE (dual-op tensor_scalar) and
ScalarE (Square + Relu trick) to balance engine load.
"""
import numpy as np
import ml_dtypes
from contextlib import ExitStack

N_COM = 40000
N_POS = 40000
D = 128
NCORES = 8
SLICE = N_COM // NCORES
TILES = 40
NT_TAB = 313
NPAD = NT_TAB * 128
EPS = 1e-20
SCAL_MOD = 3  # every SCAL_MOD-th one-hot build goes to ScalarE

RELS = [
    ("demand", "com", "pos"),
    ("cflow", "com", "com"),
    ("supply", "pos", "com"),
    ("pflow", "pos", "pos"),
]


def _prep_relation(src, dst, w, tab_bf16):
    """Host-side layout for one relation (all cores): pre-replicated row
    stream, neighbor-weight layout, per-block w/dst columns, deg_in pad."""
    src = np.asarray(src, np.int64)
    dst = np.asarray(dst, np.int64)
    w = np.asarray(w, np.float32)

    # padded-by-src weight array (for per-edge out-degree replication)
    counts_s = np.bincount(src, minlength=NPAD)
    P2 = max(8, ((int(counts_s.max()) + 7) // 8) * 8)
    deg_pad = np.zeros((NPAD, P2), ml_dtypes.bfloat16)
    order_s = np.argsort(src, kind="stable")
    ssrc, sw = src[order_s], w[order_s]
    starts = np.zeros(NPAD, np.int64)
    starts[1:] = np.cumsum(counts_s)[:-1]
    deg_pad[ssrc, np.arange(len(ssrc)) - starts[ssrc]] = sw.astype(ml_dtypes.bfloat16)

    core_of = dst // SLICE
    dloc_all = dst - core_of * SLICE
    tile_all = dloc_all // 128
    counts_grid = np.zeros((NCORES, TILES), np.int64)
    for k in range(NCORES):
        m = core_of == k
        np.add.at(counts_grid[k], tile_all[m], 1)
    blocks_tile = np.maximum(
        (np.ceil(counts_grid.max(axis=0) / 128)).astype(np.int64), 1)
    NB = int(blocks_tile.sum())

    P_in = 8
    percore_masks = []
    for k in range(NCORES):
        m = core_of == k
        percore_masks.append(m)
        cnt_in = np.bincount(dloc_all[m], minlength=5120)
        P_in = max(P_in, ((int(cnt_in.max()) + 7) // 8) * 8)

    per_core = []
    for k in range(NCORES):
        m = percore_masks[k]
        s_k, w_k = src[m], w[m]
        t_k, dl_k = tile_all[m], dloc_all[m]

        cnt_in = np.bincount(dl_k, minlength=5120)
        deg_in_pad = np.zeros((5120, P_in), np.float32)
        order_d = np.argsort(dl_k, kind="stable")
        sdl, swk = dl_k[order_d], w_k[order_d]
        st = np.zeros(5120, np.int64)
        st[1:] = np.cumsum(cnt_in)[:-1]
        deg_in_pad[sdl, np.arange(len(sdl)) - st[sdl]] = swk
        deg_in_cols = deg_in_pad.reshape(TILES, 128, P_in).transpose(1, 0, 2).reshape(
            128, TILES * P_in).astype(ml_dtypes.bfloat16)

        eidx = np.zeros(NB * 128, np.int64)   # source node per slot (pad -> 0)
        wcol = np.zeros(NB * 128, np.float32)
        dcol = np.zeros(NB * 128, np.float32)
        order = np.argsort(t_k, kind="stable")
        s_o, w_o, d_o = s_k[order], w_k[order], dl_k[order]
        t_o = t_k[order]
        starts_g = np.searchsorted(t_o, np.arange(TILES))
        ends_g = np.searchsorted(t_o, np.arange(TILES) + 1)
        off = 0
        for t in range(TILES):
            nb = int(blocks_tile[t])
            a, b = starts_g[t], ends_g[t]
            n = b - a
            eidx[off:off + n] = s_o[a:b]
            wcol[off:off + n] = w_o[a:b]
            dcol[off:off + n] = d_o[a:b] - t * 128
            off += nb * 128

        # partition-major streams: [128, NB*...]  slot p, block b = edge b*128+p
        rows = tab_bf16[eidx].reshape(NB, 128, D).transpose(1, 0, 2).reshape(
            128, NB * D).copy()
        nbrw = deg_pad[eidx].reshape(NB, 128, P2).transpose(1, 0, 2).reshape(
            128, NB * P2).copy()
        per_core.append({
            "rows": rows,
            "nbrw": nbrw,
            "wcol": wcol.reshape(NB, 128).T.copy(),
            "dcol": dcol.reshape(NB, 128).T.copy(),
            "deg_in": deg_in_cols,
        })
    return per_core, blocks_tile, P2, P_in


def _build_kernel(shapes):
    import concourse.bass as bass  # noqa: F401
    import concourse.tile as tile
    from concourse import bacc, mybir

    f32 = mybir.dt.float32
    bf16 = mybir.dt.bfloat16
    nc = bacc.Bacc("TRN2", target_bir_lowering=False, debug=False,
                   enable_asserts=False, num_devices=NCORES)

    ins, scratch = {}, {}
    for rname, s_t, d_t in RELS:
        sh = shapes[rname]
        NB = int(sh["blocks_tile"].sum())
        P2, P_in = sh["P2"], sh["P_in"]
        ins[rname] = {
            "rows": nc.dram_tensor(f"{rname}_rows", [128, NB * D], bf16,
                                   kind="ExternalInput"),
            "nbrw": nc.dram_tensor(f"{rname}_nbrw", [128, NB * P2], bf16,
                                   kind="ExternalInput"),
            "wcol": nc.dram_tensor(f"{rname}_wcol", [128, NB], f32, kind="ExternalInput"),
            "dcol": nc.dram_tensor(f"{rname}_dcol", [128, NB], f32, kind="ExternalInput"),
            "din": nc.dram_tensor(f"{rname}_degin", [128, TILES * P_in], bf16,
                                  kind="ExternalInput"),
            "W": nc.dram_tensor(f"W_{rname}", [D, D], bf16, kind="ExternalInput"),
            "b": nc.dram_tensor(f"b_{rname}", [D, 1], f32, kind="ExternalInput"),
        }
        scratch[f"{rname}_rinT"] = nc.dram_tensor(f"{rname}_rinT", [1, TILES * 128], f32)
    out = nc.dram_tensor("out", [2, D, SLICE], f32, kind="ExternalOutput")

    with tile.TileContext(nc) as tc:
        with ExitStack() as ctx:
            const_p = ctx.enter_context(tc.tile_pool(name="const", bufs=1))
            deg_p = ctx.enter_context(tc.tile_pool(name="deg", bufs=2))
            idxp = ctx.enter_context(tc.tile_pool(name="idx", bufs=1))
            gp = ctx.enter_context(tc.tile_pool(name="g", bufs=3))
            ohp = ctx.enter_context(tc.tile_pool(name="oh", bufs=6))
            sqp = ctx.enter_context(tc.tile_pool(name="sq", bufs=3))
            psp = ctx.enter_context(tc.tile_pool(name="ps", bufs=3, space="PSUM"))
            ps2 = ctx.enter_context(tc.tile_pool(name="ps2", bufs=2, space="PSUM"))
            ep = ctx.enter_context(tc.tile_pool(name="ep", bufs=6))
            rbp = ctx.enter_context(tc.tile_pool(name="rb", bufs=1))
            keep = ctx.enter_context(tc.tile_pool(name="keep", bufs=1))

            iota_i = const_p.tile([128, 128], mybir.dt.int32)
            nc.gpsimd.iota(iota_i[:], pattern=[[1, 128]], base=0, channel_multiplier=0)
            iota_f = const_p.tile([128, 128], f32)
            nc.vector.tensor_copy(iota_f[:], iota_i[:])
            pidx_i = const_p.tile([128, 1], mybir.dt.int32)
            nc.gpsimd.iota(pidx_i[:], pattern=[[1, 1]], base=0, channel_multiplier=1)
            pidx_f = const_p.tile([128, 1], f32)
            nc.vector.tensor_copy(pidx_f[:], pidx_i[:])
            ident = const_p.tile([128, 128], f32)
            nc.vector.tensor_scalar(ident[:], iota_f[:], pidx_f[:], None,
                                    op0=mybir.AluOpType.is_equal)

            acc_out = {
                "com": keep.tile([128, TILES * 128], f32, tag="acc_com", name="acc_com"),
                "pos": keep.tile([128, TILES * 128], f32, tag="acc_pos", name="acc_pos"),
            }
            first_rel = {"com": True, "pos": True}
            blk_counter = 0

            for rname, s_t, d_t in RELS:
                sh = shapes[rname]
                P2, P_in = sh["P2"], sh["P_in"]
                blocks_tile = sh["blocks_tile"]
                NB = int(blocks_tile.sum())
                inr = ins[rname]

                # --- per-edge out-degree -> rsqrt -> fold into w ---
                NBCH = 128  # blocks per chunk for the nbrw reduce
                rocol = idxp.tile([128, NB], f32, tag="rocol")
                nbv = inr["nbrw"].ap().rearrange("p (nb q) -> p nb q", q=P2)
                for c0 in range(0, NB, NBCH):
                    cn = min(NBCH, NB - c0)
                    nb_t = deg_p.tile([128, NBCH * P2], bf16, tag="nbrw")
                    dv = nb_t[:].rearrange("p (nb q) -> p nb q", q=P2)
                    nc.sync.dma_start(dv[:, 0:cn, :], nbv[:, c0:c0 + cn, :])
                    nc.vector.reduce_sum(rocol[:, c0:c0 + cn], dv[:, 0:cn, :],
                                         axis=mybir.AxisListType.X)
                nc.vector.tensor_scalar_max(rocol[:], rocol[:], EPS)
                nc.scalar.activation(rocol[:], rocol[:],
                                     mybir.ActivationFunctionType.Sqrt)
                nc.vector.reciprocal(rocol[:], rocol[:])
                wcol_t = idxp.tile([128, NB], f32, tag="wcol")
                nc.sync.dma_start(wcol_t[:], inr["wcol"].ap())
                wr = idxp.tile([128, NB], f32, tag="wr")
                nc.vector.tensor_mul(wr[:], wcol_t[:], rocol[:])
                nwr = idxp.tile([128, NB], f32, tag="nwr")
                nc.vector.tensor_scalar_mul(nwr[:], wr[:], -1.0)
                dcol_t = idxp.tile([128, NB], f32, tag="dcol")
                nc.sync.dma_start(dcol_t[:], inr["dcol"].ap())

                # --- deg_in -> rsqrt -> broadcast along partitions ---
                di_t = deg_p.tile([128, TILES * P_in], bf16, tag="din")
                nc.sync.dma_start(di_t[:], inr["din"].ap())
                r_in = deg_p.tile([128, 128], f32, tag="rin")
                nc.vector.memset(r_in[:], 1.0)
                nc.vector.reduce_sum(r_in[:, 0:TILES],
                                     di_t[:].rearrange("p (t q) -> p t q", q=P_in),
                                     axis=mybir.AxisListType.X)
                nc.vector.tensor_scalar_max(r_in[:], r_in[:], EPS)
                nc.scalar.activation(r_in[:], r_in[:],
                                     mybir.ActivationFunctionType.Sqrt)
                nc.vector.reciprocal(r_in[:], r_in[:])
                rinT_ps = ps2.tile([128, 128], f32, tag="rinT")
                nc.tensor.transpose(rinT_ps[:], r_in[:], ident[:])
                rinT = deg_p.tile([128, 128], f32, tag="rinTs")
                nc.vector.tensor_copy(rinT[0:TILES, :], rinT_ps[0:TILES, :])
                rin_hbm = scratch[f"{rname}_rinT"]
                nc.sync.dma_start(
                    rin_hbm.ap().rearrange("o (t q) -> (o t) q", q=128),
                    rinT[0:TILES, :])
                rb_row = deg_p.tile([1, TILES * 128], f32, tag="rbrow")
                nc.sync.dma_start(rb_row[:], rin_hbm.ap())
                rb_t = rbp.tile([128, TILES * 128], f32, tag="rb")
                nc.gpsimd.partition_broadcast(rb_t[:], rb_row[:])

                W_sb = const_p.tile([128, D], bf16, tag=f"W_{rname}")
                nc.sync.dma_start(W_sb[:], inr["W"].ap())
                b_col = const_p.tile([128, 1], f32, tag=f"b_{rname}")
                nc.sync.dma_start(b_col[:], inr["b"].ap())
                halfb = const_p.tile([128, 1], f32, tag=f"hb_{rname}")
                nc.scalar.activation(halfb[:], b_col[:],
                                     mybir.ActivationFunctionType.Copy, scale=0.5)

                rows_v = inr["rows"].ap().rearrange("p (nb d) -> p nb d", d=D)
                boff = 0
                for t in range(TILES):
                    nb = int(blocks_tile[t])
                    g = gp.tile([128, nb * D], bf16, tag="g")
                    gv = g[:].rearrange("p (b d) -> p b d", d=D)
                    nc.sync.dma_start(gv[:], rows_v[:, boff:boff + nb, :])
                    ps = psp.tile([128, 128], f32, tag="acc")
                    for b in range(nb):
                        col = boff + b
                        oh = ohp.tile([128, 128], bf16, tag="oh")
                        if blk_counter % 7 in (3, 6):
                            # ScalarE path: sq=(d-iota)^2; oh=relu(w-w*sq)
                            sq = sqp.tile([128, 128], f32, tag="sq")
                            nc.scalar.activation(
                                sq[:], iota_f[:],
                                mybir.ActivationFunctionType.Square,
                                bias=dcol_t[:, col:col + 1], scale=-1.0)
                            nc.scalar.activation(
                                oh[:], sq[:],
                                mybir.ActivationFunctionType.Relu,
                                bias=wr[:, col:col + 1],
                                scale=nwr[:, col:col + 1])
                        else:
                            nc.vector.tensor_scalar(
                                oh[:], iota_f[:],
                                dcol_t[:, col:col + 1], wr[:, col:col + 1],
                                op0=mybir.AluOpType.is_equal,
                                op1=mybir.AluOpType.mult)
                        blk_counter += 1
                        nc.tensor.matmul(
                            ps[:], g[:, b * D:(b + 1) * D], oh[:],
                            start=(b == 0), stop=(b == nb - 1))
                    boff += nb
                    gT = ep.tile([128, 128], bf16, tag="gT")
                    nc.scalar.activation(gT[:], ps[:],
                                         mybir.ActivationFunctionType.Copy)
                    yT_ps = ps2.tile([128, 128], f32, tag="yT")
                    nc.tensor.matmul(yT_ps[:], W_sb[:], gT[:], start=True, stop=True)
                    tmp = ep.tile([128, 128], f32, tag="tmp")
                    nc.vector.tensor_mul(tmp[:], yT_ps[:],
                                         rb_t[:, t * 128:(t + 1) * 128])
                    acc = acc_out[d_t]
                    if first_rel[d_t]:
                        nc.scalar.activation(acc[:, t * 128:(t + 1) * 128], tmp[:],
                                             mybir.ActivationFunctionType.Relu,
                                             bias=halfb[:], scale=0.5)
                    else:
                        tmp2 = ep.tile([128, 128], f32, tag="tmp2")
                        nc.scalar.activation(tmp2[:], tmp[:],
                                             mybir.ActivationFunctionType.Relu,
                                             bias=halfb[:], scale=0.5)
                        nc.vector.tensor_add(
                            acc[:, t * 128:(t + 1) * 128],
                            acc[:, t * 128:(t + 1) * 128], tmp2[:])
                first_rel[d_t] = False

            for i, ntype in enumerate(("com", "pos")):
                acc = acc_out[ntype]
                nc.sync.dma_start(out.ap()[i, :, :], acc[:, 0:SLICE])
    nc.compile()
    return nc


def kernel(**inputs):
    global LAST_RES
    from concourse.bass_utils import run_bass_kernel_spmd

    tabs = {
        "com": np.asarray(inputs["com_emb"], np.float32).astype(ml_dtypes.bfloat16),
        "pos": np.asarray(inputs["pos_emb"], np.float32).astype(ml_dtypes.bfloat16),
    }
    tabs_pad = {}
    for k, v in tabs.items():
        tp = np.zeros((NPAD, D), ml_dtypes.bfloat16)
        tp[:v.shape[0]] = v
        tabs_pad[k] = tp

    shapes, percore_rel = {}, {}
    for rname, s_t, d_t in RELS:
        per_core, blocks_tile, P2, P_in = _prep_relation(
            inputs[f"{rname}_src"], inputs[f"{rname}_dst"], inputs[f"{rname}_w"],
            tabs_pad[s_t])
        shapes[rname] = {"blocks_tile": blocks_tile, "P2": P2, "P_in": P_in}
        percore_rel[rname] = per_core

    nc = _build_kernel(shapes)

    in_maps = []
    for k in range(NCORES):
        m = {}
        for rname, s_t, d_t in RELS:
            pc = percore_rel[rname][k]
            m[f"{rname}_rows"] = pc["rows"]
            m[f"{rname}_nbrw"] = pc["nbrw"]
            m[f"{rname}_wcol"] = pc["wcol"]
            m[f"{rname}_dcol"] = pc["dcol"]
            m[f"{rname}_degin"] = pc["deg_in"]
            m[f"W_{rname}"] = np.asarray(inputs[f"W_{rname}"], np.float32).astype(
                ml_dtypes.bfloat16)
            m[f"b_{rname}"] = np.asarray(inputs[f"b_{rname}"], np.float32).reshape(D, 1)
        in_maps.append(m)

    res = run_bass_kernel_spmd(nc, in_maps, core_ids=list(range(NCORES)))
    LAST_RES = res
    out = np.empty((2, N_COM, D), np.float32)
    for k in range(NCORES):
        o = res.results[k]["out"]
        out[0, k * SLICE:(k + 1) * SLICE] = o[0].T
        out[1, k * SLICE:(k + 1) * SLICE] = o[1].T
    return out


LAST_RES = None
